# revision 1
# baseline (speedup 1.0000x reference)
"""AttentionLSTMDecoder Trainium2 kernel (8-core SPMD).

Sharding: data-parallel over batch B=64 -> 8 graphs/core for the
recurrent part (attention over that core's node segment + 2-layer LSTM),
then one AllGather of the h1 trajectories and vocab-sharded fc (each
core computes a 4000-wide vocab slice of the logits for all positions).

Matmul dtypes: float32r (fp32 data, PE-rounded, 1 cyc/row at N>=256)
for the LSTM / hp / fc paths; bf16 for the attention data path
(scores rhs, E, ctx operands) and the precomputed embedding gates.
Simulated end-to-end error vs the fp32 reference: ~5e-4 relative.

Device layout (per core, 8 local graphs):
  - feature-major tensors keep features on partitions, local graphs on
    the free axis: h.T / ctx.T are [512, 8] stored as [128, 4, 8] tiles.
  - gate tensors are [8, 2048] (graphs on partitions) so the LSTM
    weight stream is the moving matmul operand.
"""

import math

import numpy as np

B, T, H, E, V, NTOT = 64, 20, 512, 512, 32000, 8192
NCORES = 8
BL = B // NCORES          # 8 graphs per core
POS = T * BL              # 160 positions per core
VSH = V // NCORES         # 4000 vocab rows per core
VPAD = 4096               # padded vocab shard
G4 = 4 * H                # 2048 gate width
NEG = -40.0               # mask bias for off-segment scores

_COMPILED = {}


def _build(n_pad, use_b0, use_b1, use_ba):
    import concourse.bacc as bacc
    import concourse.mybir as mybir
    import concourse.tile as tile
    from contextlib import ExitStack

    f32 = mybir.dt.float32
    f32r = mybir.dt.float32r
    bf16 = mybir.dt.bfloat16
    AF = mybir.ActivationFunctionType

    nk = n_pad // 128         # node K-tiles
    nc = bacc.Bacc("TRN2", target_bir_lowering=False, debug=False,
                   num_devices=NCORES)

    D = {}
    def din(name, shape, dt=f32r):
        D[name] = nc.dram_tensor(name, shape, dt, kind="ExternalInput").ap()
        return D[name]

    nfT = din("nfT", [128, 5, n_pad], bf16)          # [NF.T; ones-row; 0] blocks
    wcT = din("wcT", [128, 5, 512], bf16)            # [W_c.T; b_c; 0] blocks
    waT = din("waT", [128, 4, 512], bf16)            # W_a.T blocks
    wcaT = din("wcaT", [128, 5, 512], bf16)    # [W_c.T@W_a; b_c@W_a; 0] blocks
    baR = din("baR", [128, 4], f32)            # b_a per M-chunk
    msk = din("msk", [128, n_pad], bf16)       # mask rhs block
    mi8 = din("mi8", [128, 8], bf16)           # mask lhsT block (I8 + ones row)
    i8r = din("i8r", [8, 8])                   # identity (f32r)
    i8b = din("i8b", [8, 8], bf16)             # identity (bf16)
    embT = din("embT", [128, 4, POS], bf16)          # emb.T blocks, cols t*8+b
    wembT = din("wembT", [128, 4, G4], bf16)         # W_ih0[:, :512].T blocks
    b0c = din("b0c", [128, G4], f32)           # b0 broadcast (only if used)
    w0T = din("w0T", [128, 8, G4])             # [W_ctx.T; W_hh0.T] blocks
    w1T = din("w1T", [128, 8, G4])             # [W_ih1.T; W_hh1.T] blocks
    b1r = din("b1r", [8, G4], bf16)            # b1 rows (only if used)
    gfT = din("gfT", [128, 4, 8])
    gfTb = din("gfTb", [128, 4, 8], bf16)              # graph_features.T blocks
    wfcT = din("wfcT", [32, 128, 4, 128])      # fc weights per V-tile
    bfc = din("bfc", [128, 32], f32)           # b_fc (partition=m, free=vt)
    out_d = nc.dram_tensor("out", [32, 128, NCORES * POS], f32,
                           kind="ExternalOutput").ap()

    with tile.TileContext(nc) as tc, ExitStack() as ctx:
        res = ctx.enter_context(tc.tile_pool(name="res", bufs=1))
        dram = ctx.enter_context(tc.tile_pool(name="dram", bufs=1, space="DRAM"))
        drsh = ctx.enter_context(tc.tile_pool(name="drsh", bufs=1, space="DRAM"))

        npT = res.tile([128, 5, n_pad], bf16, tag="npT")   # [NP.T blocks; mask]
        npB = res.tile([128, nk, 512], bf16, tag="npB")    # NP node-major blocks
        was = res.tile([128, 5 - use_ba, 4 if use_ba else 512, 512]
                       if False else
                       ([128, 4, 512] if use_ba else [128, 5, 512]),
                       bf16, tag="was")
        bas = res.tile([128, 4], f32, tag="bas")
        i8rs = res.tile([8, 8], f32r, tag="i8rs")
        i8bs = res.tile([8, 8], bf16, tag="i8bs")
        hall = res.tile([128, 4, POS], f32r, tag="hall")
        h1T = res.tile([128, 4, 8], f32r, tag="h1T")
        h1Tb = res.tile([128, 4, 8], bf16, tag="h1Tb")
        x0T = res.tile([128, 8, 8], f32r, tag="x0T")       # [ctx.T | h0.T]
        h0T = x0T[:, 4:8, :]
        c0s = res.tile([8, H], f32, tag="c0s")
        c1s = res.tile([8, H], f32, tag="c1s")
        hpT = res.tile([128, 5, 8], bf16, tag="hpT")
        b1s = res.tile([8, G4], bf16, tag="b1s") if use_b1 else None

        nc.sync.dma_start(was[:], waT[:] if use_ba else wcaT[:])
        nc.sync.dma_start(bas[:], baR[:])
        nc.sync.dma_start(i8rs[:], i8r[:])
        nc.sync.dma_start(i8bs[:], i8b[:])
        nc.sync.dma_start(npT[:, 4, :], msk[:])
        nc.sync.dma_start(hpT[:, 4, :], mi8[:])
        nc.sync.dma_start(h1T[:], gfT[:])
        nc.sync.dma_start(h1Tb[:], gfTb[:])
        nc.sync.dma_start(x0T[:, 4:8, :], gfT[:])
        nc.gpsimd.memset(c0s[:], 0.0)
        nc.gpsimd.memset(c1s[:], 0.0)
        if use_b1:
            nc.sync.dma_start(b1s[:], b1r[:])

        eg_dram = dram.tile([POS, G4], bf16)

        lstm_ctx = ExitStack()
        lstm = lstm_ctx.enter_context(tc.tile_pool(name="lstm", bufs=1))
        w0s = lstm.tile([128, 8, G4], f32r, tag="w0s")
        w1s = lstm.tile([128, 8, G4], f32r, tag="w1s")

        # ---------------- phase A: NP.T, NP, EG0 ----------------
        with tc.tile_pool(name="pha", bufs=1, side="right") as pha, \
             tc.tile_pool(name="phap", bufs=1, space="PSUM") as phap:
            nfs = pha.tile([128, 5, n_pad], bf16, tag="nfs")
            wcs = pha.tile([128, 5, 512], bf16, tag="wcs")
            nc.sync.dma_start(nfs[:], nfT[:])
            nc.sync.dma_start(wcs[:], wcT[:])
            nc.sync.dma_start(w0s[:], w0T[:])
            nc.sync.dma_start(w1s[:], w1T[:])

            # NP.T chunk mt = sum_kt wcs[:,kt,mt-chunk].T @ nfs[:,kt,:]
            # (kt==4 adds b_c via the ones-row)
            for mt in range(4):
                p = phap.tile([128, n_pad], f32, tag="pa")
                for kt in range(5):
                    lt = (wcs if use_ba else was)[:, kt, mt * 128:(mt + 1) * 128]
                    for c0 in range(0, n_pad, 512):
                        cw = min(512, n_pad - c0)
                        nc.tensor.matmul(
                            p[:, c0:c0 + cw], lt,
                            nfs[:, kt, c0:c0 + cw],
                            start=(kt == 0), stop=(kt == 4))
                nc.scalar.copy(npT[:, mt, :], p[:])

            # NP block j = sum_kt nfs[:,kt,j-chunk].T @ wcs[:,kt,:]
            for j in range(nk):
                p = phap.tile([128, 512], f32, tag="pb")
                for kt in range(5):
                    nc.tensor.matmul(
                        p[:], nfs[:, kt, j * 128:(j + 1) * 128],
                        wcs[:, kt, :], start=(kt == 0), stop=(kt == 4))
                nc.scalar.copy(npB[:, j, :], p[:])

            # EG0 [POS, 2048] = embT.T @ wembT (+ b0)
            ems = pha.tile([128, 4, POS], bf16, tag="ems")
            nc.sync.dma_start(ems[:], embT[:])
            if use_b0:
                b0s = pha.tile([128, G4], f32, tag="b0s")
                nc.sync.dma_start(b0s[:], b0c[:])
            for mc in range(0, POS, 128):
                mw = min(128, POS - mc)
                p = phap.tile([128, G4], f32, tag="pc")
                for c0 in range(0, G4, 512):
                    wes = pha.tile([128, 4, 512], bf16, tag="wes", bufs=2)
                    nc.sync.dma_start(wes[:], wembT[:, :, c0:c0 + 512])
                    for kt in range(4):
                        nc.tensor.matmul(
                            p[:mw, c0:c0 + 512],
                            ems[:, kt, mc:mc + mw],
                            wes[:, kt, :],
                            start=(kt == 0), stop=(kt == 3))
                for h0_ in (0, 1024):
                    eo = pha.tile([128, 1024], bf16, tag="eo")
                    if use_b0:
                        nc.vector.tensor_add(eo[:mw, :], p[:mw, h0_:h0_ + 1024],
                                             b0s[:mw, h0_:h0_ + 1024])
                    else:
                        nc.scalar.copy(eo[:mw, :], p[:mw, h0_:h0_ + 1024])
                    nc.sync.dma_start(eg_dram[mc:mc + mw, h0_:h0_ + 1024],
                                      eo[:mw, :])

        # ---------------- recurrence ----------------
        with tc.tile_pool(name="stepp", bufs=1) as stepp, \
             tc.tile_pool(name="ctxp", bufs=2) as ctxp, \
             tc.tile_pool(name="egp", bufs=2) as egp, \
             tc.tile_pool(name="big", bufs=1, space="PSUM") as big, \
             tc.tile_pool(name="scp", bufs=1, space="PSUM") as scp, \
             tc.tile_pool(name="sml", bufs=1, space="PSUM") as sml:

            NCH = (T + 3) // 4
            ag_ins = [dram.tile([512, 32], f32r, tag=f"agi{i}", name=f"agi{i}")
                      for i in range(NCH)]
            ag_outs = [drsh.tile([NCORES * 512, 32], f32r,
                                 addr_space="Shared", tag=f"ago{i}",
                                 name=f"ago{i}")
                       for i in range(NCH)]
            def cell(gP, cS, dsts):
                """LSTM cell from gates psum gP [8,2048]; writes the
                transposed new hidden state into each dst [128,4,8]."""
                cw = stepp.tile([8, G4], f32, tag="cw")
                nc.scalar.activation(cw[:, 0:1024], gP[:, 0:1024], AF.Sigmoid)
                nc.scalar.activation(cw[:, 1024:1536], gP[:, 1024:1536], AF.Tanh)
                nc.scalar.activation(cw[:, 1536:2048], gP[:, 1536:2048], AF.Sigmoid)
                t1 = stepp.tile([8, 512], f32, tag="t1")
                nc.vector.tensor_mul(t1[:], cw[:, 0:512], cw[:, 1024:1536])
                t2 = stepp.tile([8, 512], f32, tag="t2")
                nc.vector.tensor_mul(t2[:], cw[:, 512:1024], cS[:])
                nc.vector.tensor_add(cS[:], t1[:], t2[:])
                tch = stepp.tile([8, 512], f32, tag="tch")
                nc.scalar.activation(tch[:], cS[:], AF.Tanh)
                hn = stepp.tile([8, 512], f32r, tag="hn")
                nc.vector.tensor_mul(hn[:], cw[:, 1536:2048], tch[:])
                tp = sml.tile([128, 32], f32r, tag="s1")
                for j in range(4):
                    nc.tensor.transpose(tp[:, j * 8:(j + 1) * 8],
                                        hn[:, j * 128:(j + 1) * 128], i8rs[:])
                tpv = tp[:].rearrange("p (a b) -> p a b", a=4)
                for dst in dsts:
                    nc.vector.tensor_copy(dst, tpv)

            for t in range(T):
                eg = egp.tile([8, G4], bf16, tag="eg")
                nc.sync.dma_start(eg[:], eg_dram[t * 8:(t + 1) * 8, :])

                if use_ba:
                    # hp.T = W_a @ h1.T (+ b_a)
                    hpP = sml.tile([128, 32], f32, tag="s1")
                    for mt in range(4):
                        for kt in range(4):
                            nc.tensor.matmul(hpP[:, mt * 8:(mt + 1) * 8],
                                             was[:, kt, mt * 128:(mt + 1) * 128],
                                             h1Tb[:, kt, :],
                                             start=(kt == 0), stop=(kt == 3))
                    for mt in range(4):
                        nc.scalar.activation(hpT[:, mt, :],
                                             hpP[:, mt * 8:(mt + 1) * 8],
                                             AF.Identity, bias=bas[:, mt:mt + 1])

                # scores S.T [8, n_pad] = hp8 @ NP.T + mask
                stP = scp.tile([8, n_pad], f32, tag="scp")
                for kt in range(5):
                    lt = hpT[:, kt, :] if (use_ba or kt == 4) else h1Tb[:, kt, :]
                    for c0 in range(0, n_pad, 512):
                        cw_ = min(512, n_pad - c0)
                        nc.tensor.matmul(stP[:, c0:c0 + cw_], lt,
                                         npT[:, kt, c0:c0 + cw_],
                                         start=(kt == 0), stop=(kt == 4))
                Et = stepp.tile([8, n_pad], bf16, tag="Et")
                den = stepp.tile([8, 1], f32, tag="den")
                nc.scalar.activation(Et[:], stP[:, 0:n_pad], AF.Exp,
                                     accum_out=den[:])
                r8 = stepp.tile([8, 1], f32, tag="r8")
                nc.vector.reciprocal(r8[:], den[:])

                # E.T via PE transposes (bf16)
                etP = sml.tile([128, nk * 8], bf16, tag="s1")
                for j in range(nk):
                    nc.tensor.transpose(etP[:, j * 8:(j + 1) * 8],
                                        Et[:, j * 128:(j + 1) * 128], i8bs[:])
                etT = stepp.tile([128, nk, 8], bf16, tag="etT")
                nc.vector.tensor_copy(etT[:],
                                      etP[:].rearrange("p (a b) -> p a b", a=nk))

                # ctx [8, 512] = E @ NP, scaled by 1/den on copy-out
                ctxP = sml.tile([8, 512], f32, tag="s1")
                for j in range(nk):
                    nc.tensor.matmul(ctxP[:], etT[:, j, :], npB[:, j, :],
                                     start=(j == 0), stop=(j == nk - 1))
                ctxS = ctxp.tile([8, 512], f32r, tag="ctxS")
                nc.scalar.activation(ctxS[:], ctxP[:], AF.Copy, scale=r8[:])

                # ctx.T -> x0T[:, 0:4, :]
                ctP = sml.tile([128, 32], f32r, tag="s1")
                for j in range(4):
                    nc.tensor.transpose(ctP[:, j * 8:(j + 1) * 8],
                                        ctxS[:, j * 128:(j + 1) * 128], i8rs[:])
                nc.vector.tensor_copy(x0T[:, 0:4, :],
                                      ctP[:].rearrange("p (a b) -> p a b", a=4))

                # gates0 = [ctx|h0].T.T @ w0 + EG0[t]
                g0P = big.tile([8, 2048], f32, tag="big")
                for c0 in range(0, G4, 512):
                    for kt in (4, 5, 6, 7, 0, 1, 2, 3):
                        nc.tensor.matmul(g0P[:, c0:c0 + 512], x0T[:, kt, :],
                                         w0s[:, kt, c0:c0 + 512],
                                         start=(kt == 4), stop=False)
                    nc.tensor.matmul(g0P[:, c0:c0 + 512], i8bs[:],
                                     eg[:, c0:c0 + 512], start=False, stop=True)
                cell(g0P, c0s, [h0T])

                # gates1 = [h0n|h1].T.T @ w1 (+ b1)
                g1P = big.tile([8, 2048], f32, tag="big")
                for c0 in range(0, G4, 512):
                    for kt in (4, 5, 6, 7, 0, 1, 2, 3):
                        src = h1T[:, kt - 4, :] if kt >= 4 else h0T[:, kt, :]
                        nc.tensor.matmul(g1P[:, c0:c0 + 512], src,
                                         w1s[:, kt, c0:c0 + 512],
                                         start=(kt == 4),
                                         stop=(kt == 3 and not use_b1))
                    if use_b1:
                        nc.tensor.matmul(g1P[:, c0:c0 + 512], i8bs[:],
                                         b1s[:, c0:c0 + 512],
                                         start=False, stop=True)
                cell(g1P, c1s,
                     [h1Tb[:], h1T[:], hall[:, :, t * 8:(t + 1) * 8]])

                if t % 4 == 3:
                    ch = t // 4
                    agi = ag_ins[ch]
                    nc.sync.dma_start(
                        agi[:].rearrange("(a p) n -> p a n", p=128),
                        hall[:, :, ch * 32:(ch + 1) * 32])
                    nc.gpsimd.collective_compute(
                        "AllGather", mybir.AluOpType.bypass,
                        replica_groups=[list(range(NCORES))],
                        ins=[agi.opt()], outs=[ag_outs[ch].opt()])

        lstm_ctx.close()

        # ---------------- fc (vocab shard) ----------------
        NPOS = NCORES * POS  # 1280
        with tc.tile_pool(name="fcp", bufs=1) as fcp, \
             tc.tile_pool(name="fco", bufs=3) as fco, \
             tc.tile_pool(name="fcps", bufs=2, space="PSUM") as fcps:
            NCH = (T + 3) // 4
            hA = [fcp.tile([128, 4, 256], f32r, tag=f"hA{ch}",
                           name=f"hA{ch}") for ch in range(NCH)]
            for ch in range(NCH):
                for c in range(NCORES):
                    nc.sync.dma_start(
                        hA[ch][:, :, c * 32:(c + 1) * 32],
                        ag_outs[ch][c * 512:(c + 1) * 512].rearrange(
                            "(a p) n -> p a n", p=128))
            bfcs = fcp.tile([128, 32], f32, tag="bfcs")
            nc.sync.dma_start(bfcs[:], bfc[:])
            wts = [fcp.tile([128, 4, 128], f32r, tag=f"wt{vt}",
                            name=f"wt{vt}") for vt in range(32)]
            for vt in range(32):
                nc.sync.dma_start(wts[vt][:], wfcT[vt])
            # pass 1: position chunks 0..NCH-2 (AG chunks already landed)
            for vt in range(32):
                p = fcps.tile([128, (NCH - 1) * 256], f32, tag="fp1")
                for ch in range(NCH - 1):
                    for kt in range(4):
                        nc.tensor.matmul(p[:, ch * 256:(ch + 1) * 256],
                                         wts[vt][:, kt, :],
                                         hA[ch][:, kt, :],
                                         start=(kt == 0), stop=(kt == 3))
                ot = fco.tile([128, (NCH - 1) * 256], f32, tag="ot")
                nc.scalar.activation(ot[:], p[:], AF.Identity,
                                     bias=bfcs[:, vt:vt + 1])
                nc.sync.dma_start(out_d[vt][:, 0:(NCH - 1) * 256], ot[:])
            # pass 2: the last chunk (waits only on the final AllGather)
            for vt in range(32):
                p = fcps.tile([128, 256], f32, tag="fp2")
                for kt in range(4):
                    nc.tensor.matmul(p[:], wts[vt][:, kt, :],
                                     hA[NCH - 1][:, kt, :],
                                     start=(kt == 0), stop=(kt == 3))
                ot = fco.tile([128, 256], f32, tag="ot2")
                nc.scalar.activation(ot[:], p[:], AF.Identity,
                                     bias=bfcs[:, vt:vt + 1])
                nc.sync.dma_start(out_d[vt][:, (NCH - 1) * 256:NPOS], ot[:])

    nc.compile()
    return nc


def _prep(inputs, n_pad):
    import ml_dtypes
    gf = np.ascontiguousarray(np.asarray(inputs["graph_features"], np.float32))
    nf = np.ascontiguousarray(np.asarray(inputs["node_features"], np.float32))
    emb = np.asarray(inputs["embedding"], np.float32)
    W_a = np.asarray(inputs["W_a"], np.float32)
    b_a = np.asarray(inputs["b_a"], np.float32)
    W_c = np.asarray(inputs["W_c"], np.float32)
    b_c = np.asarray(inputs["b_c"], np.float32)
    W_ih0 = np.asarray(inputs["W_ih0"], np.float32)
    W_hh0 = np.asarray(inputs["W_hh0"], np.float32)
    b0 = np.asarray(inputs["b_ih0"], np.float32) + np.asarray(inputs["b_hh0"], np.float32)
    W_ih1 = np.asarray(inputs["W_ih1"], np.float32)
    W_hh1 = np.asarray(inputs["W_hh1"], np.float32)
    b1 = np.asarray(inputs["b_ih1"], np.float32) + np.asarray(inputs["b_hh1"], np.float32)
    W_fc = np.asarray(inputs["W_fc"], np.float32)
    b_fc = np.asarray(inputs["b_fc"], np.float32)
    bidx = np.asarray(inputs["batch_idx"]).astype(np.int64)
    caps = np.asarray(inputs["captions"]).astype(np.int64)
    bf = ml_dtypes.bfloat16

    def blocks(a):
        K, N = a.shape
        return np.ascontiguousarray(a.reshape(K // 128, 128, N).transpose(1, 0, 2))

    wcT_full = np.zeros((640, 512), np.float32)
    wcT_full[:512] = W_c.T
    wcT_full[512] = b_c
    waT_b = blocks(np.ascontiguousarray(W_a.T))
    wca_full = np.zeros((640, 512), np.float32)
    wca_full[:512] = W_c.T @ W_a
    wca_full[512] = b_c @ W_a
    wcaT_b = blocks(wca_full).astype(bf)
    baR = np.ascontiguousarray(b_a.reshape(4, 128).T)
    w0 = np.concatenate([W_ih0[:, 512:].T, W_hh0.T], 0)
    w1 = np.concatenate([W_ih1.T, W_hh1.T], 0)
    wembT = blocks(np.ascontiguousarray(W_ih0[:, :512].T))
    i8 = np.eye(8, dtype=np.float32)
    mi8 = np.zeros((128, 8), np.float32)
    mi8[:8, :8] = np.eye(8)
    mi8[8, :] = 1.0
    b0c = np.tile(b0[None, :], (128, 1)).astype(np.float32)
    b1r = np.tile(b1[None, :], (8, 1)).astype(bf)
    use_b0 = bool(np.any(b0 != 0))
    use_b1 = bool(np.any(b1 != 0))

    maps = []
    for k in range(NCORES):
        sel = (bidx >= k * BL) & (bidx < (k + 1) * BL)
        nodes = np.nonzero(sel)[0]
        cnt = len(nodes)
        nfT_full = np.zeros((640, n_pad), np.float32)
        nfT_full[:512, :cnt] = nf[nodes].T
        nfT_full[512, :cnt] = 1.0
        lb = bidx[nodes] - k * BL
        msk = np.zeros((128, n_pad), np.float32)
        msk[8, :] = NEG
        msk[lb, np.arange(cnt)] = -NEG
        e = emb[caps[k * BL:(k + 1) * BL]]             # [8, T, E]
        embT_full = np.ascontiguousarray(e.transpose(2, 1, 0).reshape(E, POS))
        wfc = np.zeros((VPAD, H), np.float32)
        wfc[:VSH] = W_fc[k * VSH:(k + 1) * VSH]
        wfcT_b = np.ascontiguousarray(
            wfc.reshape(32, 128, 4, 128).transpose(0, 3, 2, 1))
        bfc_b = np.zeros((VPAD,), np.float32)
        bfc_b[:VSH] = b_fc[k * VSH:(k + 1) * VSH]
        bfc_b = np.ascontiguousarray(bfc_b.reshape(32, 128).T)
        m = {
            "nfT": blocks(nfT_full).astype(bf),
            "wcT": blocks(wcT_full).astype(bf),
            "waT": waT_b.astype(bf), "wcaT": wcaT_b, "baR": baR,
            "msk": msk.astype(bf), "mi8": mi8.astype(bf),
            "i8r": i8, "i8b": i8.astype(bf),
            "embT": blocks(embT_full).astype(bf),
            "wembT": wembT.astype(bf), "b0c": b0c,
            "w0T": blocks(w0), "w1T": blocks(w1), "b1r": b1r,
            "gfT": blocks(np.ascontiguousarray(gf[k * BL:(k + 1) * BL].T)),
            "gfTb": blocks(np.ascontiguousarray(gf[k * BL:(k + 1) * BL].T)).astype(bf),
            "wfcT": wfcT_b, "bfc": bfc_b,
        }
        maps.append(m)
    return maps, use_b0, use_b1


def kernel(**inputs) -> np.ndarray:
    from concourse.bass_utils import run_bass_kernel_spmd

    bidx = np.asarray(inputs["batch_idx"]).astype(np.int64)
    counts = np.bincount(bidx // BL, minlength=NCORES)
    n_pad = max(256, int(math.ceil(counts.max() / 128.0)) * 128)
    maps, use_b0, use_b1 = _prep(inputs, n_pad)
    use_ba = bool(np.any(np.asarray(inputs["b_a"], np.float32) != 0))
    key = (n_pad, use_b0, use_b1, use_ba)
    if key not in _COMPILED:
        _COMPILED[key] = _build(n_pad, use_b0, use_b1, use_ba)
    res = run_bass_kernel_spmd(_COMPILED[key], maps,
                               core_ids=list(range(NCORES)))
    out = np.empty((B, T, V), np.float32)
    nch = (T + 3) // 4
    for k in range(NCORES):
        o = res.results[k]["out"].reshape(VPAD, nch, NCORES, T // nch, BL)
        # -> [c, b, ch, dt, v] -> graph-major, t = ch*4+dt
        o = o[:VSH].transpose(2, 4, 1, 3, 0).reshape(B, T, VSH)
        out[:, :, k * VSH:(k + 1) * VSH] = o
    return out



# revision 2
# speedup vs baseline: 1.1046x; 1.1046x over previous
"""AttentionLSTMDecoder Trainium2 kernel v2 (8-core SPMD).

Sharding: data-parallel over batch B=64 -> 8 graphs/core for the
recurrent part (attention over that core's node segment + 2-layer LSTM),
AllGather of h1 trajectories in 6 step-chunks, vocab-sharded fc whose
matmuls are interleaved into the recurrence as PE filler work.

Numerics: bf16 operands everywhere with fp32 PSUM accumulation.
sigmoid(x) = 0.5 + 0.5*tanh(x/2) so the scalar engine never switches
activation tables (exp/tanh share one set).  Hidden states are stored
as h' = 2h with all h-consuming weights pre-halved on the host, which
absorbs the 0.5 factors of the tanh-based sigmoid.

Cell math runs feature-major ([128, 4, 8] tiles) on the vector engine;
the activated gates are transposed on the PE (cheap 8-col transposes).
"""

import math

import numpy as np

B, T, H, E, V, NTOT = 64, 20, 512, 512, 32000, 8192
NCORES = 8
BL = B // NCORES          # 8 graphs per core
POS = T * BL              # 160 positions per core
VSH = V // NCORES         # 4000 vocab rows per core
VPAD = 4096               # padded vocab shard
G4 = 4 * H                # 2048 gate width
GW = 512                  # gate matmul free width
FW = 512                  # fc matmul free width

CH_SIZES = [2, 2, 4, 4, 2, 2, 2, 2]
CH_STARTS = [0, 2, 4, 8, 12, 14, 16, 18]
FC_CAP = 5                # max filler tasks emitted per step

_COMPILED = {}


def _pos_tiles():
    """Position-tile metadata: list of (chunk, hA column offset)."""
    pt, ch_off, off = [], [], 0
    for S in CH_SIZES:
        ch_off.append(off)
        npos = NCORES * S * BL
        for pc in range(npos // 128):
            pt.append((len(ch_off) - 1, off + pc * 128))
        off += npos
    return pt, ch_off


def _build(n_pad, use_b0, use_b1, use_ba, use_bc, use_bfc):
    import concourse.bacc as bacc
    import concourse.mybir as mybir
    import concourse.tile as tile
    from contextlib import ExitStack

    f32 = mybir.dt.float32
    bf16 = mybir.dt.bfloat16
    AF = mybir.ActivationFunctionType

    nk = n_pad // 128
    KC = 5 if use_bc else 4
    pt_meta, ch_off = _pos_tiles()
    NPT = len(pt_meta)        # 10 position tiles of 128

    nc = bacc.Bacc("TRN2", target_bir_lowering=False, debug=False,
                   num_devices=NCORES)

    def din(name, shape, dt=bf16):
        return nc.dram_tensor(name, shape, dt, kind="ExternalInput").ap()

    nfT = din("nfT", [128, KC, n_pad])       # node_features.T blocks (+ones)
    wcaT = din("wcaT", [128, KC, 512])       # (W_c.T @ W_a)/2 blocks
    wcT = din("wcT", [128, KC, 512])         # W_c.T blocks
    msk = din("msk", [128, nk, 8])           # 0/1 segment mask, node-major
    i8b = din("i8b", [8, 8])                 # identity
    i8p = din("i8p", [8, 128])               # identity padded to 128 cols
    embT = din("embT", [128, 4, POS])        # emb.T cols t*8+b
    wembT = din("wembT", [128, 4, G4])       # W_ih0[:, :512].T blocks
    w0T = din("w0T", [128, 8, G4])           # [W_ctx.T; W_hh0.T/2] blocks
    w1T = din("w1T", [128, 8, G4])           # [W_ih1.T/2; W_hh1.T/2] blocks
    wfcT = din("wfcT", [128, 4, VPAD])       # (W_fc-shard/2).T blocks
    gfT2 = din("gfT2", [128, 4, 8])          # 2*graph_features.T blocks
    b0c = din("b0c", [128, G4], f32) if use_b0 else None
    b1r = din("b1r", [8, G4]) if use_b1 else None
    sbias = din("sbias", [128, nk, 8], f32) if use_ba else None
    bfcr = din("bfcr", [128, VPAD], f32) if use_bfc else None
    out_d = nc.dram_tensor("out", [NPT, 128, VPAD], bf16,
                           kind="ExternalOutput").ap()

    # Filler task schedule. fc tasks become available two steps after
    # their chunk's AllGather was issued (AG latency margin); eg tasks
    # (deferred EG0 row-groups) fill the fc-less early steps.
    fc_sched = {t: [] for t in range(T)}
    pending = []
    for grp in range(1, POS // 32):
        for c0 in range(0, G4, 512):
            pending.append((max(0, grp - 2), ("eg", grp, c0)))
    for tau, (ch, poff) in enumerate(pt_meta):
        avail = CH_STARTS[ch] + CH_SIZES[ch] - 1 + 3
        for v0 in range(0, VPAD, FW):
            pending.append((avail, ("fc", tau, poff, v0)))
    pending.sort(key=lambda x: x[0])
    for t in range(T):
        n = 0
        rest = []
        for avail, task in pending:
            if avail <= t and n < FC_CAP:
                fc_sched[t].append(task)
                n += 1
            else:
                rest.append((avail, task))
        pending = rest
    tail_tasks = [task for _, task in pending]

    with tile.TileContext(nc) as tc, ExitStack() as ctx:
        res = ctx.enter_context(tc.tile_pool(name="res", bufs=1))
        dram = ctx.enter_context(tc.tile_pool(name="dram", bufs=1, space="DRAM"))
        drsh = ctx.enter_context(tc.tile_pool(name="drsh", bufs=1, space="DRAM"))

        npT = res.tile([128, 4, n_pad], bf16, tag="npT")
        npB = res.tile([128, nk, 512], bf16, tag="npB")
        msks = res.tile([128, nk, 8], bf16, tag="msks")
        i8bs = res.tile([8, 8], bf16, tag="i8bs")
        i8ps = res.tile([8, 128], bf16, tag="i8ps")
        ones = res.tile([128, 1], bf16, tag="ones")
        w0s = res.tile([128, 8, G4], bf16, tag="w0s")
        w1s = res.tile([128, 8, G4], bf16, tag="w1s")
        wfcs = res.tile([128, 4, VPAD], bf16, tag="wfcs")
        hA = res.tile([128, 4, NCORES * POS], bf16, tag="hA")
        hall = res.tile([128, 4, POS], bf16, tag="hall")
        # stationary operands padded to 128 columns: NumWeights==128
        # enables the PE fast-weight-load path (~44ns/MM saved)
        x0T = res.tile([128, 8, 128], bf16, tag="x0T")  # [ctx.T | h0']
        h1s = res.tile([128, 4, 128], bf16, tag="h1s")  # h1'
        EtM = res.tile([128, nk, 128], bf16, tag="EtM")
        c0s = res.tile([128, 4, 8], f32, tag="c0s")
        c1s = res.tile([128, 4, 8], f32, tag="c1s")
        b1s = res.tile([8, G4], bf16, tag="b1s") if use_b1 else None
        sbs = res.tile([128, nk, 8], f32, tag="sbs") if use_ba else None
        bfcs = res.tile([128, VPAD], f32, tag="bfcs") if use_bfc else None

        nc.sync.dma_start(msks[:], msk[:])
        nc.sync.dma_start(i8bs[:], i8b[:])
        nc.sync.dma_start(i8ps[:], i8p[:])
        nc.gpsimd.memset(x0T[:], 0.0)
        nc.gpsimd.memset(h1s[:], 0.0)
        nc.gpsimd.memset(EtM[:], 0.0)
        nc.sync.dma_start(x0T[:, 4:8, 0:8], gfT2[:])
        nc.sync.dma_start(h1s[:, :, 0:8], gfT2[:])
        nc.gpsimd.memset(ones[:], 1.0)
        nc.gpsimd.memset(c0s[:], 0.0)
        nc.gpsimd.memset(c1s[:], 0.0)
        if use_b1:
            nc.sync.dma_start(b1s[:], b1r[:])
        if use_ba:
            nc.sync.dma_start(sbs[:], sbias[:])
        if use_bfc:
            nc.sync.dma_start(bfcs[:], bfcr[:])

        eg_dram = dram.tile([POS, G4], bf16)
        ems = res.tile([128, 4, POS], bf16, tag="ems")
        wes = res.tile([128, 4, G4], bf16, tag="wes")
        b0s = res.tile([128, G4], f32, tag="b0s") if use_b0 else None

        # ---------------- phase A: npT, npB, EG0 rows t0-3 ----------------
        with tc.tile_pool(name="pha", bufs=1, side="right") as pha, \
             tc.tile_pool(name="phap", bufs=2, space="PSUM") as phap:
            nfs = pha.tile([128, KC, n_pad], bf16, tag="nfs")
            wcas = pha.tile([128, KC, 512], bf16, tag="wcas")
            wcs = pha.tile([128, KC, 512], bf16, tag="wcs")
            nc.sync.dma_start(nfs[:], nfT[:])
            nc.sync.dma_start(wcas[:], wcaT[:])
            nc.sync.dma_start(wcs[:], wcT[:])
            nc.sync.dma_start(ems[:], embT[:])
            nc.sync.dma_start(wes[:], wembT[:])
            # big resident loads, split per k-block so step-0 consumers
            # wait only for the slices they touch
            for kt in range(8):
                nc.sync.dma_start(w0s[:, kt, :], w0T[:, kt, :])
            for kt in range(8):
                nc.sync.dma_start(w1s[:, kt, :], w1T[:, kt, :])
            for kt in range(4):
                nc.sync.dma_start(wfcs[:, kt, :], wfcT[:, kt, :])
            if use_b0:
                nc.sync.dma_start(b0s[:], b0c[:])

            # npT block mt = sum_kt wcas[:,kt,mt-chunk].T @ nfs[:,kt,:]
            for mt in range(4):
                for c0 in range(0, n_pad, 512):
                    cw = min(512, n_pad - c0)
                    p = phap.tile([128, 512], f32, tag="pa")
                    for kt in range(KC):
                        nc.tensor.matmul(
                            p[:, :cw], wcas[:, kt, mt * 128:(mt + 1) * 128],
                            nfs[:, kt, c0:c0 + cw],
                            start=(kt == 0), stop=(kt == KC - 1))
                    nc.vector.tensor_copy(npT[:, mt, c0:c0 + cw], p[:, :cw])
            # npB block j = nfs[:,kt,j-chunk].T @ wcs
            for j in range(nk):
                p = phap.tile([128, 512], f32, tag="pb")
                for kt in range(KC):
                    nc.tensor.matmul(
                        p[:], nfs[:, kt, j * 128:(j + 1) * 128], wcs[:, kt, :],
                        start=(kt == 0), stop=(kt == KC - 1))
                nc.vector.tensor_copy(npB[:, j, :], p[:])
            # EG0 rows for steps 0-3 only; later row-groups are emitted
            # inside steps 0-2 as PE filler (see eg_task below)
            for c0 in range(0, G4, 512):
                p = phap.tile([128, 512], f32, tag="pa")
                for kt in range(4):
                    nc.tensor.matmul(
                        p[:32, :], ems[:, kt, 0:32],
                        wes[:, kt, c0:c0 + 512],
                        start=(kt == 0), stop=(kt == 3))
                eo = pha.tile([128, 512], bf16, tag="eo", bufs=2)
                if use_b0:
                    nc.vector.tensor_add(eo[:32, :], p[:32, :],
                                         b0s[:32, c0:c0 + 512])
                else:
                    nc.vector.tensor_copy(eo[:32, :], p[:32, :])
                nc.sync.dma_start(eg_dram[0:32, c0:c0 + 512], eo[:32, :])

        # warmup collective: absorbs the one-time CC init latency (~11us)
        # while phase A is still computing
        agw_i = dram.tile([8, 8], bf16, tag="agwi", name="agw_i")
        agw_o = drsh.tile([64, 8], bf16, addr_space="Shared", tag="agwo",
                          name="agw_o")
        nc.sync.dma_start(agw_i[:], i8b[:])
        nc.gpsimd.collective_compute(
            "AllGather", mybir.AluOpType.bypass,
            replica_groups=[list(range(NCORES))],
            ins=[agw_i.opt()], outs=[agw_o.opt()])

        # ---------------- recurrence + interleaved fc ----------------
        ag_ins, ag_outs = [], []
        for ch, S in enumerate(CH_SIZES):
            w = S * BL
            agi = dram.tile([4 * 128, w], bf16, tag=f"agi{ch}", name=f"agi{ch}")
            ago = drsh.tile([NCORES * 512, w], bf16, addr_space="Shared",
                            tag=f"ago{ch}", name=f"ago{ch}")
            ag_ins.append(agi)
            ag_outs.append(ago)

        with tc.tile_pool(name="stp", bufs=1) as stp, \
             tc.tile_pool(name="egp", bufs=2) as egp, \
             tc.tile_pool(name="gp", bufs=1, space="PSUM") as gp, \
             tc.tile_pool(name="smp", bufs=1, space="PSUM") as smp, \
             tc.tile_pool(name="ctxpp", bufs=1, space="PSUM") as ctxpp, \
             tc.tile_pool(name="fcp", bufs=2, space="PSUM") as fcp:

            def fc_emit(task):
                if task[0] == "eg":
                    _, grp, c0 = task
                    r0 = grp * 32
                    p = fcp.tile([32, 512], f32, tag="fcP", name="egP")
                    for kt in range(4):
                        nc.tensor.matmul(p[:], ems[:, kt, r0:r0 + 32],
                                         wes[:, kt, c0:c0 + 512],
                                         start=(kt == 0), stop=(kt == 3))
                    eo = stp.tile([32, 512], bf16, tag="eo", bufs=2)
                    if use_b0:
                        nc.vector.tensor_add(eo[:], p[:],
                                             b0s[0:32, c0:c0 + 512])
                    else:
                        nc.vector.tensor_copy(eo[:], p[:])
                    nc.sync.dma_start(eg_dram[r0:r0 + 32, c0:c0 + 512], eo[:])
                    return
                _, tau, poff, v0 = task
                p = fcp.tile([128, FW], f32, tag="fcP")
                for kt in range(4):
                    nc.tensor.matmul(p[:], hA[:, kt, poff:poff + 128],
                                     wfcs[:, kt, v0:v0 + FW],
                                     start=(kt == 0), stop=(kt == 3))
                ot = stp.tile([128, FW], bf16, tag="fco", bufs=2)
                if use_bfc:
                    nc.vector.tensor_add(ot[:], p[:], bfcs[:, v0:v0 + FW])
                else:
                    nc.vector.tensor_copy(ot[:], p[:])
                nc.sync.dma_start(out_d[tau][:, v0:v0 + FW], ot[:])

            def gates_part(gP, w_s, kis, x_aps, hoff, first, last,
                           eg_ap=None, b1_ap=None):
                # one K-subset of a gates half; first/last flag the psum
                # accumulation-group boundaries for each 512-chunk
                for c0 in range(0, 1024, GW):
                    seg = slice(hoff + c0, hoff + c0 + GW)
                    extra = (eg_ap is not None) or (b1_ap is not None)
                    for n, ki in enumerate(kis):
                        nc.tensor.matmul(gP[:, c0:c0 + GW], x_aps[n],
                                         w_s[:, ki, seg],
                                         start=(first and n == 0),
                                         stop=(last and not extra
                                               and n == len(kis) - 1))
                    if eg_ap is not None:
                        nc.tensor.matmul(gP[:, c0:c0 + GW], i8ps[:],
                                         eg_ap[:, seg], start=False,
                                         stop=(last and b1_ap is None))
                    if b1_ap is not None:
                        nc.tensor.matmul(gP[:, c0:c0 + GW], i8ps[:],
                                         b1_ap[:, seg], start=False, stop=last)

            def nl_A(gP, dst, tg):
                # i,f gates: sigma~ = tanh(g/2); transpose to feature-major
                At = stp.tile([8, 1024], bf16, tag=tg, name="At")
                nc.scalar.activation(At[:], gP[0:8, :], AF.Tanh, scale=0.5)
                tr = smp.tile([128, 64], bf16, tag="sm", name="trA")
                for q in range(8):
                    nc.tensor.transpose(tr[:, q * 8:(q + 1) * 8],
                                        At[:, q * 128:(q + 1) * 128], i8bs[:])
                nc.vector.tensor_copy(dst[:],
                                      tr[:].rearrange("p (a b) -> p a b", a=8))

            def _tr8(src, dst, nq, nm):
                tr = smp.tile([128, 8 * nq], bf16, tag="sm", name=nm)
                for q in range(nq):
                    nc.tensor.transpose(tr[:, q * 8:(q + 1) * 8],
                                        src[:, q * 128:(q + 1) * 128], i8bs[:])
                nc.vector.tensor_copy(
                    dst[:], tr[:].rearrange("p (a b) -> p a b", a=nq))

            def cell_pre(tif, cS):
                # zc = (c + tf*c) + tg*(1 + ti); this is the tf/ti part,
                # issued while the B-half gates are still on the PE
                ti, tf = tif[:, 0:4, :], tif[:, 4:8, :]
                m = stp.tile([128, 4, 8], f32, tag="m")
                nc.vector.tensor_mul(m[:], tf, cS[:])
                u1 = stp.tile([128, 4, 8], f32, tag="u1")
                nc.vector.tensor_add(u1[:], cS[:], m[:])
                i1 = stp.tile([128, 4, 8], f32, tag="i1")
                nc.vector.tensor_scalar_add(i1[:], ti, 1.0)
                return u1, i1

            def cell_post(gP, u1, i1, cS, h_dst, tg, hook=None):
                # g gate first so the c-chain starts before the o gate's ACT
                Btg = stp.tile([8, 512], bf16, tag=tg + "g", name="Btg")
                nc.scalar.activation(Btg[:], gP[0:8, 0:512], AF.Tanh)
                Bto = stp.tile([8, 512], bf16, tag=tg + "o", name="Bto")
                nc.scalar.activation(Bto[:], gP[0:8, 512:1024], AF.Tanh,
                                     scale=0.5)
                if hook is not None:
                    hook()   # gP fully consumed: safe to recycle its slot
                tgs = stp.tile([128, 4, 8], bf16, tag="tgs")
                _tr8(Btg, tgs, 4, "trg")
                v = stp.tile([128, 4, 8], f32, tag="v")
                nc.vector.tensor_mul(v[:], tgs[:], i1[:])
                zc = stp.tile([128, 4, 8], f32, tag="zc")
                nc.vector.tensor_add(zc[:], u1[:], v[:])
                tcv = stp.tile([128, 4, 8], bf16, tag="tcv")
                nc.scalar.activation(tcv[:], zc[:], AF.Tanh, scale=0.5)
                tos = stp.tile([128, 4, 8], bf16, tag="tos")
                _tr8(Bto, tos, 4, "tro")
                ho = stp.tile([128, 4, 8], bf16, tag="ho")
                nc.vector.tensor_mul(ho[:], tos[:], tcv[:])
                nc.vector.tensor_add(h_dst, tcv[:], ho[:])
                nc.vector.tensor_scalar_mul(cS[:], zc[:], 0.5)

            for t in range(T):
                fcq = list(fc_sched.get(t, []))

                def fcpop(k=1):
                    for _ in range(k):
                        if fcq:
                            fc_emit(fcq.pop(0))

                eg = egp.tile([8, G4], bf16, tag="eg")
                nc.sync.dma_start(eg[:], eg_dram[t * 8:(t + 1) * 8, :])

                # gates0 h0-part first: depends only on h0'(t-1), so these
                # MMs fill the PE while cell1(t-1)'s chain finishes
                g0A = gp.tile([128, 1024], f32, tag="gA", name="g0A")
                g0B = gp.tile([128, 1024], f32, tag="gB", name="g0B")
                h0_aps = [x0T[:, 4 + k, :] for k in range(4)]
                gates_part(g0A, w0s, [4, 5, 6, 7], h0_aps, 0, True, False)
                gates_part(g0B, w0s, [4, 5, 6, 7], h0_aps, 1024, True, False)

                # scores.T [128, nk, 8] = npT.T-blocks @ h1'
                scT = smp.tile([128, nk, 8], f32, tag="sm", name="scT")
                for j in range(nk):
                    for kt in range(4):
                        nc.tensor.matmul(scT[:, j, :],
                                         npT[:, kt, j * 128:(j + 1) * 128],
                                         h1s[:, kt, 0:8],
                                         start=(kt == 0), stop=(kt == 3))
                fcpop()
                Et = stp.tile([128, nk, 8], bf16, tag="Et")
                if use_ba:
                    sS = stp.tile([128, nk, 8], f32, tag="sS")
                    nc.vector.tensor_add(sS[:], scT[:], sbs[:])
                    nc.scalar.activation(Et[:], sS[:], AF.Exp)
                else:
                    nc.scalar.activation(Et[:], scT[:], AF.Exp)
                nc.vector.tensor_mul(EtM[:, :, 0:8], Et[:], msks[:])
                # ctx [8,512] and den [8,1]
                ctxP = ctxpp.tile([128, 512], f32, tag="ctxP")
                denP = smp.tile([8, 8], f32, tag="sm", name="denP")
                for j in range(nk):
                    nc.tensor.matmul(ctxP[:], EtM[:, j, :], npB[:, j, :],
                                     start=(j == 0), stop=(j == nk - 1))
                    nc.tensor.matmul(denP[:, 0:1], EtM[:, j, 0:8],
                                     ones[:, 0:1],
                                     start=(j == 0), stop=(j == nk - 1))
                r8 = stp.tile([8, 1], f32, tag="r8")
                nc.vector.reciprocal(r8[:], denP[:, 0:1])
                ctxS = stp.tile([8, 512], bf16, tag="ctxS")
                nc.scalar.activation(ctxS[:], ctxP[0:8, :], AF.Copy,
                                     scale=r8[:])
                fcpop()
                trc = smp.tile([128, 32], bf16, tag="sm", name="trc")
                for q in range(4):
                    nc.tensor.transpose(trc[:, q * 8:(q + 1) * 8],
                                        ctxS[:, q * 128:(q + 1) * 128], i8bs[:])
                nc.vector.tensor_copy(x0T[:, 0:4, 0:8],
                                      trc[:].rearrange("p (a b) -> p a b", a=4))

                # gates0 ctx-part + cell0 (writes h0' into x0T[:, 4:8])
                ctx_aps = [x0T[:, k, :] for k in range(4)]
                b1a = b1s if use_b1 else None
                gates_part(g0A, w0s, [0, 1, 2, 3], ctx_aps, 0, False, True,
                           eg_ap=eg)
                gates_part(g0B, w0s, [0, 1, 2, 3], ctx_aps, 1024, False, True,
                           eg_ap=eg)
                tif0 = stp.tile([128, 8, 8], bf16, tag="tif")
                nl_A(g0A, tif0, "At0")
                u1, i1 = cell_pre(tif0, c0s)
                # gates1 h1-part: depends only on h1'(t-1); fills cell0's chain
                g1A = gp.tile([128, 1024], f32, tag="gA", name="g1A")
                h1_aps = [h1s[:, k, :] for k in range(4)]
                gates_part(g1A, w1s, [4, 5, 6, 7], h1_aps, 0, True, False)
                fcpop()
                g1B_box = []

                def _g1b_hook():
                    g1B = gp.tile([128, 1024], f32, tag="gB", name="g1B")
                    gates_part(g1B, w1s, [4, 5, 6, 7], h1_aps, 1024, True,
                               False)
                    g1B_box.append(g1B)

                cell_post(g0B, u1, i1, c0s, x0T[:, 4:8, 0:8], "Bt0",
                          hook=_g1b_hook)
                g1B = g1B_box[0]
                fcpop()
                # gates1 h0-part (needs the fresh h0')
                h0n_aps = [x0T[:, 4 + k, :] for k in range(4)]
                gates_part(g1A, w1s, [0, 1, 2, 3], h0n_aps, 0, False, True,
                           b1_ap=b1a)
                gates_part(g1B, w1s, [0, 1, 2, 3], h0n_aps, 1024, False, True,
                           b1_ap=b1a)
                tif1 = stp.tile([128, 8, 8], bf16, tag="tif")
                nl_A(g1A, tif1, "At1")
                u1b, i1b = cell_pre(tif1, c1s)
                fcpop()
                cell_post(g1B, u1b, i1b, c1s, h1s[:, :, 0:8], "Bt1")
                nc.vector.tensor_copy(hall[:, :, t * 8:(t + 1) * 8],
                                      h1s[:, :, 0:8])
                fcpop()

                for ch, (st, S) in enumerate(zip(CH_STARTS, CH_SIZES)):
                    if t == st + S - 1:
                        agi, ago = ag_ins[ch], ag_outs[ch]
                        nc.sync.dma_start(
                            agi[:].rearrange("(a p) n -> p a n", p=128),
                            hall[:, :, st * 8:(st + S) * 8])
                        nc.gpsimd.collective_compute(
                            "AllGather", mybir.AluOpType.bypass,
                            replica_groups=[list(range(NCORES))],
                            ins=[agi.opt()], outs=[ago.opt()])
                        w = S * BL
                        for c in range(NCORES):
                            nc.sync.dma_start(
                                hA[:, :, ch_off[ch] + c * w:
                                   ch_off[ch] + (c + 1) * w],
                                ago[c * 512:(c + 1) * 512].rearrange(
                                    "(a p) n -> p a n", p=128))
                # leftover fc tasks for this step
                fcpop(len(fcq))

            for task in tail_tasks:
                fc_emit(task)

    nc.compile()
    return nc


def _prep(inputs, n_pad):
    import ml_dtypes
    bf = ml_dtypes.bfloat16
    nk = n_pad // 128

    gf = np.asarray(inputs["graph_features"], np.float32)
    nf = np.asarray(inputs["node_features"], np.float32)
    emb = np.asarray(inputs["embedding"], np.float32)
    W_a = np.asarray(inputs["W_a"], np.float32)
    b_a = np.asarray(inputs["b_a"], np.float32)
    W_c = np.asarray(inputs["W_c"], np.float32)
    b_c = np.asarray(inputs["b_c"], np.float32)
    W_ih0 = np.asarray(inputs["W_ih0"], np.float32)
    W_hh0 = np.asarray(inputs["W_hh0"], np.float32)
    b0 = (np.asarray(inputs["b_ih0"], np.float32)
          + np.asarray(inputs["b_hh0"], np.float32))
    W_ih1 = np.asarray(inputs["W_ih1"], np.float32)
    W_hh1 = np.asarray(inputs["W_hh1"], np.float32)
    b1 = (np.asarray(inputs["b_ih1"], np.float32)
          + np.asarray(inputs["b_hh1"], np.float32))
    W_fc = np.asarray(inputs["W_fc"], np.float32)
    b_fc = np.asarray(inputs["b_fc"], np.float32)
    bidx = np.asarray(inputs["batch_idx"]).astype(np.int64)
    caps = np.asarray(inputs["captions"]).astype(np.int64)

    use_b0 = bool(np.any(b0 != 0))
    use_b1 = bool(np.any(b1 != 0))
    use_ba = bool(np.any(b_a != 0))
    use_bc = bool(np.any(b_c != 0))
    use_bfc = bool(np.any(b_fc != 0))
    KC = 5 if use_bc else 4

    def blocks(a):
        K, N = a.shape
        return np.ascontiguousarray(
            a.reshape(K // 128, 128, N).transpose(1, 0, 2))

    def kfull(mat, brow):
        # [512(+128), N] with optional ones-row-driven bias row
        if not use_bc:
            return mat
        out = np.zeros((640, mat.shape[1]), np.float32)
        out[:512] = mat
        out[512] = brow
        return out

    wca = (W_c.T @ W_a) * 0.5
    wcaT_b = blocks(kfull(wca, (b_c @ W_a) * 0.5)).astype(bf)
    wcT_b = blocks(kfull(W_c.T, b_c)).astype(bf)
    w0 = np.concatenate([W_ih0[:, 512:].T, W_hh0.T * 0.5], 0)
    w1 = np.concatenate([W_ih1.T * 0.5, W_hh1.T * 0.5], 0)
    w0_b = blocks(w0).astype(bf)
    w1_b = blocks(w1).astype(bf)
    wembT_b = blocks(np.ascontiguousarray(W_ih0[:, :512].T)).astype(bf)
    i8 = np.eye(8, dtype=np.float32).astype(bf)
    i8pad = np.zeros((8, 128), np.float32)
    i8pad[:8, :8] = np.eye(8)
    i8pad = i8pad.astype(bf)
    b0c = np.tile(b0[None, :], (128, 1)).astype(np.float32)
    b1r = np.tile(b1[None, :], (8, 1)).astype(bf)

    maps = []
    for k in range(NCORES):
        sel = (bidx >= k * BL) & (bidx < (k + 1) * BL)
        nodes = np.nonzero(sel)[0]
        cnt = len(nodes)
        nfT_full = np.zeros((640 if use_bc else 512, n_pad), np.float32)
        nfT_full[:512, :cnt] = nf[nodes].T
        if use_bc:
            nfT_full[512, :cnt] = 1.0
        lb = bidx[nodes] - k * BL
        m01 = np.zeros((n_pad, 8), np.float32)
        m01[np.arange(cnt), lb] = 1.0
        m01 = np.ascontiguousarray(
            m01.reshape(nk, 128, 8).transpose(1, 0, 2))
        e = emb[caps[k * BL:(k + 1) * BL]]             # [8, T, E]
        embT_full = np.ascontiguousarray(
            e.transpose(2, 1, 0).reshape(E, POS))
        wfc = np.zeros((VPAD, H), np.float32)
        wfc[:VSH] = W_fc[k * VSH:(k + 1) * VSH] * 0.5
        wfcT_b = blocks(np.ascontiguousarray(wfc.T))
        m = {
            "nfT": blocks(nfT_full).astype(bf),
            "wcaT": wcaT_b, "wcT": wcT_b,
            "msk": m01.astype(bf), "i8b": i8, "i8p": i8pad,
            "embT": blocks(embT_full).astype(bf),
            "wembT": wembT_b,
            "w0T": w0_b, "w1T": w1_b,
            "wfcT": wfcT_b.astype(bf),
            "gfT2": blocks(
                np.ascontiguousarray(2.0 * gf[k * BL:(k + 1) * BL].T)
            ).astype(bf),
        }
        if use_b0:
            m["b0c"] = b0c
        if use_b1:
            m["b1r"] = b1r
        if use_ba:
            sb = (nf[nodes] @ W_c.T + b_c) @ b_a       # [cnt]
            sbf = np.zeros((n_pad,), np.float32)
            sbf[:cnt] = sb
            m["sbias"] = np.ascontiguousarray(
                np.tile(sbf.reshape(nk, 128, 1), (1, 1, 8)
                        ).transpose(1, 0, 2)).astype(np.float32)
        if use_bfc:
            bfp = np.zeros((VPAD,), np.float32)
            bfp[:VSH] = b_fc[k * VSH:(k + 1) * VSH]
            m["bfcr"] = np.tile(bfp[None, :], (128, 1)).astype(np.float32)
        maps.append(m)
    return maps, (use_b0, use_b1, use_ba, use_bc, use_bfc)


def _unshard(results):
    out = np.empty((B, T, V), np.float32)
    for k in range(NCORES):
        o = np.asarray(results[k]["out"], np.float32)  # [NPT, 128, VPAD]
        tau = 0
        for ch, (st, S) in enumerate(zip(CH_STARTS, CH_SIZES)):
            ntile = NCORES * S * BL // 128
            rows = o[tau:tau + ntile].reshape(NCORES, S, BL, VPAD)
            out[:, st:st + S, k * VSH:(k + 1) * VSH] = (
                rows[:, :, :, :VSH].transpose(0, 2, 1, 3).reshape(B, S, VSH))
            tau += ntile
    return out


def kernel(**inputs) -> np.ndarray:
    from concourse.bass_utils import run_bass_kernel_spmd

    bidx = np.asarray(inputs["batch_idx"]).astype(np.int64)
    counts = np.bincount(bidx // BL, minlength=NCORES)
    n_pad = max(256, int(math.ceil(counts.max() / 128.0)) * 128)
    maps, flags = _prep(inputs, n_pad)
    key = (n_pad,) + flags
    if key not in _COMPILED:
        _COMPILED[key] = _build(n_pad, *flags)
    res = run_bass_kernel_spmd(_COMPILED[key], maps,
                               core_ids=list(range(NCORES)))
    return _unshard(res.results)
